# revision 10
# baseline (speedup 1.0000x reference)
"""Neural CDE forward pass on 8 Trainium2 NeuronCores (Bass/Tile).

Math (per batch element b):
    z0 = u0 @ Wi + bi                                   [64]
    for t in 0..164:
        h  = relu(z @ W1 + b1)                          [128]
        f  = tanh(h @ W2 + b2)                          [512] -> [64, 8]
        z += einsum('hi,i->h', f, dx_t)                 dx_t = coeffs[t+1]-coeffs[t]
    out_t = z_t @ Wr + br  for every t (166 values)

Kernel design (per core, batch shard B=512 on the matmul free dim):
  - State zT [64, B] lives persistently in one PSUM bank; it is updated by
    accumulating matmuls (start=False) and never cleared.
  - h:  hT = W1.T @ zT            (lhsT = W1 [64,128], 1 matmul, N=512)
  - f:  fT = W2_j.T @ hT, j=0..3  (4 matmuls into a 4-bank PSUM tile)
  - tanh with per-partition bias b2_j on ScalarE -> f_sbuf
  - einsum: g_j = f_j * dx_rep  elementwise on VectorE, where
    dx_rep[p, b] = dx[b, p % 8] (host pre-replicates into DRAM, DMA streamed);
    then z_psum += S_j.T @ g_j with selection matrices S_j [128, 65]:
    S_j[p, 16j + p//8] = 1.
  - Readout is fused for free: column 64 of S_j is S_j @ Wr, so PSUM
    partition 64 accumulates out_t = z_t@Wr + br across steps; one tiny DMA
    per step stores it to DRAM.
"""

import numpy as np

IN_CH = 8
HID = 64
MLP_W = 128
OUT = 1
B_FULL, T = 4096, 166
NSTEP = T - 1
N_CORES = 8
B = B_FULL // N_CORES  # 512
NBANK = 4  # f feature banks of 128

_CACHE = {}


def _build_bass():
    from contextlib import ExitStack

    import concourse.bass as bass
    import concourse.tile as tile
    from concourse import bacc, mybir

    f32 = mybir.dt.float32
    AF = mybir.ActivationFunctionType

    nc = bacc.Bacc("TRN2", target_bir_lowering=False, debug=False)

    u0t = nc.dram_tensor("u0t", [IN_CH, B], f32, kind="ExternalInput")
    dxt = nc.dram_tensor("dxt", [NSTEP, 128, B], f32, kind="ExternalInput")
    w1 = nc.dram_tensor("w1", [HID, MLP_W], f32, kind="ExternalInput")
    b1 = nc.dram_tensor("b1", [MLP_W, 1], f32, kind="ExternalInput")
    w2 = nc.dram_tensor("w2", [MLP_W, NBANK, 128], f32, kind="ExternalInput")
    b2 = nc.dram_tensor("b2", [128, NBANK], f32, kind="ExternalInput")
    wi = nc.dram_tensor("wi", [IN_CH, HID + 1], f32, kind="ExternalInput")
    smat = nc.dram_tensor("smat", [128, NBANK, HID + 1], f32, kind="ExternalInput")
    outp = nc.dram_tensor("outp", [T, B], f32, kind="ExternalOutput")

    with tile.TileContext(nc) as tc, ExitStack() as ctx:
        const = ctx.enter_context(tc.tile_pool(name="const", bufs=1))
        zpool = ctx.enter_context(tc.tile_pool(name="zpool", bufs=2))
        hpool = ctx.enter_context(tc.tile_pool(name="hpool", bufs=2))
        fpool = ctx.enter_context(tc.tile_pool(name="fpool", bufs=2))
        gpool = ctx.enter_context(tc.tile_pool(name="gpool", bufs=2))
        dxpool = ctx.enter_context(tc.tile_pool(name="dxpool", bufs=4))
        psum_h = ctx.enter_context(tc.tile_pool(name="psum_h", bufs=2, space="PSUM"))
        psum_f = ctx.enter_context(tc.tile_pool(name="psum_f", bufs=1, space="PSUM"))
        psum_e = ctx.enter_context(tc.tile_pool(name="psum_e", bufs=2, space="PSUM"))

        w1_sb = const.tile([HID, MLP_W], f32)
        nc.sync.dma_start(w1_sb[:], w1[:])
        b1_sb = const.tile([MLP_W, 1], f32)
        nc.sync.dma_start(b1_sb[:], b1[:])
        w2_sb = const.tile([MLP_W, NBANK, 128], f32)
        nc.sync.dma_start(w2_sb[:], w2[:])
        b2_sb = const.tile([128, NBANK], f32)
        nc.sync.dma_start(b2_sb[:], b2[:])
        wi_sb = const.tile([IN_CH, HID + 1], f32)
        nc.sync.dma_start(wi_sb[:], wi[:])
        s_sb = const.tile([128, NBANK, HID + 1], f32)
        nc.sync.dma_start(s_sb[:], smat[:])
        u0t_sb = const.tile([IN_CH, B], f32)
        nc.sync.dma_start(u0t_sb[:], u0t[:])

        # State zT lives in SBUF: rows 0..63 = zT, row 64 = running readout
        # (out_t = z_t @ Wr + br), updated by z_new = z_old + e each step.
        z0_ps = psum_e.tile([HID + 1, B], f32, tag="e_ps")
        nc.tensor.matmul(z0_ps[:], wi_sb[:], u0t_sb[:], start=True, stop=True)
        z_sb = zpool.tile([HID + 1, B], f32)
        nc.vector.tensor_copy(z_sb[:], z0_ps[:])
        nc.sync.dma_start(outp[0:1, :], z_sb[HID : HID + 1, :])

        for t in range(NSTEP):
            dx_sb = dxpool.tile([128, B], f32)
            nc.sync.dma_start(dx_sb[:], dxt[t])

            h_ps = psum_h.tile([MLP_W, B], f32)
            nc.tensor.matmul(h_ps[:], w1_sb[:], z_sb[0:HID, :], start=True, stop=True)
            h_sb = hpool.tile([MLP_W, B], f32)
            nc.scalar.activation(h_sb[:], h_ps[:], AF.Relu, bias=b1_sb[:, 0:1])

            f_ps = psum_f.tile([128, NBANK * B], f32)
            for j in range(NBANK):
                nc.tensor.matmul(
                    f_ps[:, j * B : (j + 1) * B], w2_sb[:, j, :], h_sb[:],
                    start=True, stop=True,
                )
            f_sb = fpool.tile([128, NBANK * B], f32)
            for j in range(NBANK):
                nc.scalar.activation(
                    f_sb[:, j * B : (j + 1) * B], f_ps[:, j * B : (j + 1) * B],
                    AF.Tanh, bias=b2_sb[:, j : j + 1],
                )
            g_sb = gpool.tile([128, NBANK * B], f32)
            for j in range(NBANK):
                nc.vector.tensor_mul(
                    g_sb[:, j * B : (j + 1) * B], f_sb[:, j * B : (j + 1) * B],
                    dx_sb[:],
                )
            e_ps = psum_e.tile([HID + 1, B], f32, tag="e_ps")
            for j in range(NBANK):
                nc.tensor.matmul(
                    e_ps[:], s_sb[:, j, :], g_sb[:, j * B : (j + 1) * B],
                    start=j == 0, stop=j == NBANK - 1,
                )
            z_prev = z_sb
            z_sb = zpool.tile([HID + 1, B], f32)
            nc.vector.tensor_add(z_sb[:], e_ps[:], z_prev[:])
            nc.sync.dma_start(outp[t + 1 : t + 2, :], z_sb[HID : HID + 1, :])

    nc.compile()
    return nc


def _prep_host(u0, coeffs, W1, b1, W2, b2, Wi, bi, Wr, br):
    f32 = np.float32

    u0t_full = np.empty((IN_CH, B_FULL), f32)
    u0t_full[: IN_CH - 1] = u0.T
    u0t_full[IN_CH - 1] = 1.0

    dX = (coeffs[:, 1:] - coeffs[:, :-1]).astype(f32)  # [B_FULL, NSTEP, IN_CH]
    dxt_small = np.ascontiguousarray(dX.transpose(1, 2, 0))  # [NSTEP, 8, B_FULL]
    dxt_full = np.tile(dxt_small, (1, 128 // IN_CH, 1))  # [NSTEP, 128, B_FULL]

    wi_mat = np.empty((IN_CH, HID + 1), f32)
    wi_mat[: IN_CH - 1, :HID] = Wi
    wi_mat[IN_CH - 1, :HID] = bi
    wi_mat[: IN_CH - 1, HID] = (Wi @ Wr)[:, 0]
    wi_mat[IN_CH - 1, HID] = float(bi @ Wr[:, 0] + br[0])

    w2_banks = np.ascontiguousarray(W2.reshape(MLP_W, NBANK, 128))
    b2_banks = np.ascontiguousarray(b2.reshape(NBANK, 128).T)

    p = np.arange(128)
    s_full = np.zeros((128, NBANK, HID + 1), f32)
    for j in range(NBANK):
        s_full[p, j, 16 * j + p // IN_CH] = 1.0
        s_full[p, j, HID] = Wr[16 * j + p // IN_CH, 0]

    return {
        "u0t": u0t_full,
        "dxt": dxt_full,
        "w1": np.ascontiguousarray(W1.astype(f32)),
        "b1": np.ascontiguousarray(b1.astype(f32).reshape(MLP_W, 1)),
        "w2": w2_banks.astype(f32),
        "b2": b2_banks.astype(f32),
        "wi": wi_mat,
        "smat": s_full,
    }


def kernel(u0, coeffs, W1, b1, W2, b2, Wi, bi, Wr, br):
    from concourse.bass_utils import run_bass_kernel_spmd

    full = _prep_host(
        np.asarray(u0, np.float32), np.asarray(coeffs, np.float32),
        np.asarray(W1, np.float32), np.asarray(b1, np.float32),
        np.asarray(W2, np.float32), np.asarray(b2, np.float32),
        np.asarray(Wi, np.float32), np.asarray(bi, np.float32),
        np.asarray(Wr, np.float32).reshape(HID, OUT),
        np.asarray(br, np.float32).reshape(OUT),
    )

    in_maps = []
    for c in range(N_CORES):
        sl = slice(c * B, (c + 1) * B)
        in_maps.append(
            {
                "u0t": np.ascontiguousarray(full["u0t"][:, sl]),
                "dxt": np.ascontiguousarray(full["dxt"][:, :, sl]),
                "w1": full["w1"],
                "b1": full["b1"],
                "w2": full["w2"],
                "b2": full["b2"],
                "wi": full["wi"],
                "smat": full["smat"],
            }
        )

    if "nc" not in _CACHE:
        _CACHE["nc"] = _build_bass()
    nc = _CACHE["nc"]

    res = run_bass_kernel_spmd(nc, in_maps, core_ids=list(range(N_CORES)))
    outs = res.results

    out_full = np.empty((B_FULL, T, OUT), np.float32)
    for c in range(N_CORES):
        out_full[c * B : (c + 1) * B, :, 0] = outs[c]["outp"].T
    return out_full


# revision 14
# speedup vs baseline: 42.1660x; 42.1660x over previous
"""Neural CDE forward pass on 8 Trainium2 NeuronCores (Bass/Tile).

Math (per batch element b):
    z0 = u0 @ Wi + bi                                   [64]
    for t in 0..164:
        h  = relu(z @ W1 + b1)                          [128]
        f  = tanh(h @ W2 + b2)                          [512] -> [64, 8]
        z += einsum('hi,i->h', f, dx_t)                 dx_t = coeffs[t+1]-coeffs[t]
    out_t = z_t @ Wr + br  for every t (166 values)

The scan is chaotic: perturbations amplify ~1e4x over the 165 steps, so
every matmul must run in exact fp32 (bf16/f32r weights give 10%+ final
error). fp32 matmuls stream at 1/4 rate on trn2 PE and their cost is
~4*N cycles, independent of K and M; the design therefore minimizes the
number of N=512 matmul slots per step.

Kernel design (per core, batch shard B=512 on the matmul free dim):
  - Split state: z = z_a + z_b, stacked as z_stack [128, B] fp32 in SBUF
    (rows 0..63 = z_a, 64..127 = z_b). The split lets the einsum-reduce
    matmuls run as 2 column-tiled PAIRS (concurrent 64-wide outputs)
    instead of 4 serial M=65 matmuls, and mm1 contracts the sum for free:
  - h:  h_ps = [W1;W1].T @ z_stack     (K=128, one matmul slot)
  - f:  fT = W2_j.T @ h, j=0..3        (4 slots into one 4-bank PSUM tile)
  - tanh per bank on ScalarE with fused per-partition bias b2_j.
  - einsum: g_j = f_j * dx_rep elementwise fp32 (VectorE + GpSimd),
    dx_rep[p, b] = dx[b, p % 8] (host pre-replicated, DMA streamed);
    e_ps [128, B]: rows 0..63  = S0.T g_0 + S2.T g_2   (col group 0-1)
                   rows 64..127= S1.T g_1 + S3.T g_3   (col group 2-3)
    via 2 slots of column-tiled matmul pairs, S_j [128, 64] selection
    matrices: S_j[p, 16j + p//8] = 1. Then z_stack += e_ps (one DVE add).
  - readout: wz = z_stack * [Wr;Wr] (per-partition scale, VectorE), then
    GpSimd partition_all_reduce sums all 128 partitions = z @ Wr; row 0
    is DMA'd to DRAM. br is added on the host.
"""

import numpy as np

IN_CH = 8
HID = 64
MLP_W = 128
OUT = 1
B_FULL, T = 4096, 166
NSTEP = T - 1
N_CORES = 8
B = B_FULL // N_CORES  # 512
NBANK = 4  # f feature banks of 128

# tuning knobs
RELU_ON = "act"  # "act" | "dve"
G_ON_GPSIMD = 1  # how many of the 4 g-multiplies run on GpSimd

_CACHE = {}


def _build_bass():
    from contextlib import ExitStack

    import concourse.bass_isa as bass_isa
    import concourse.tile as tile
    from concourse import bacc, mybir

    f32 = mybir.dt.float32
    AF = mybir.ActivationFunctionType
    ALU = mybir.AluOpType

    nc = bacc.Bacc("TRN2", target_bir_lowering=False, debug=False)

    u0t = nc.dram_tensor("u0t", [IN_CH, B], f32, kind="ExternalInput")
    dxt = nc.dram_tensor("dxt", [NSTEP, 128, B], f32, kind="ExternalInput")
    w1s = nc.dram_tensor("w1s", [MLP_W, MLP_W], f32, kind="ExternalInput")
    b1 = nc.dram_tensor("b1", [MLP_W, 1], f32, kind="ExternalInput")
    w2 = nc.dram_tensor("w2", [MLP_W, NBANK, 128], f32, kind="ExternalInput")
    b2 = nc.dram_tensor("b2", [128, NBANK], f32, kind="ExternalInput")
    wi = nc.dram_tensor("wi", [IN_CH, MLP_W], f32, kind="ExternalInput")
    smat = nc.dram_tensor("smat", [128, NBANK, HID], f32, kind="ExternalInput")
    wrs = nc.dram_tensor("wrs", [MLP_W, 1], f32, kind="ExternalInput")
    outp = nc.dram_tensor("outp", [T, B], f32, kind="ExternalOutput")

    with tile.TileContext(nc) as tc, ExitStack() as ctx:
        const = ctx.enter_context(tc.tile_pool(name="const", bufs=1))
        zpool = ctx.enter_context(tc.tile_pool(name="zpool", bufs=2))
        hpool = ctx.enter_context(tc.tile_pool(name="hpool", bufs=2))
        fpool = ctx.enter_context(tc.tile_pool(name="fpool", bufs=2))
        gpool = ctx.enter_context(tc.tile_pool(name="gpool", bufs=2))
        wzpool = ctx.enter_context(tc.tile_pool(name="wzpool", bufs=2))
        dxpool = ctx.enter_context(tc.tile_pool(name="dxpool", bufs=4))
        psum_h = ctx.enter_context(tc.tile_pool(name="psum_h", bufs=2, space="PSUM"))
        psum_f = ctx.enter_context(tc.tile_pool(name="psum_f", bufs=1, space="PSUM"))
        psum_e = ctx.enter_context(tc.tile_pool(name="psum_e", bufs=2, space="PSUM"))

        w1s_sb = const.tile([MLP_W, MLP_W], f32)
        nc.sync.dma_start(w1s_sb[:], w1s[:])
        b1_sb = const.tile([MLP_W, 1], f32)
        nc.sync.dma_start(b1_sb[:], b1[:])
        w2_sb = const.tile([MLP_W, NBANK, 128], f32)
        nc.sync.dma_start(w2_sb[:], w2[:])
        b2_sb = const.tile([128, NBANK], f32)
        nc.sync.dma_start(b2_sb[:], b2[:])
        wi_sb = const.tile([IN_CH, MLP_W], f32)
        nc.sync.dma_start(wi_sb[:], wi[:])
        s_sb = const.tile([128, NBANK, HID], f32)
        nc.sync.dma_start(s_sb[:], smat[:])
        wrs_sb = const.tile([MLP_W, 1], f32)
        nc.sync.dma_start(wrs_sb[:], wrs[:])
        u0t_sb = const.tile([IN_CH, B], f32)
        nc.sync.dma_start(u0t_sb[:], u0t[:])

        def readout(z_stack, row):
            wz = wzpool.tile([MLP_W, B], f32, tag="wz")
            nc.vector.tensor_scalar_mul(wz[:], z_stack[:], wrs_sb[:, 0:1])
            red = wzpool.tile([MLP_W, B], f32, tag="red")
            nc.gpsimd.partition_all_reduce(
                red[:], wz[:], MLP_W, bass_isa.ReduceOp.add
            )
            nc.sync.dma_start(outp[row : row + 1, :], red[0:1, :])

        # init: z_a = z0 (rows 0..63), z_b = 0 (rows 64..127)
        z0_ps = psum_e.tile([MLP_W, B], f32, tag="e_ps")
        nc.tensor.matmul(z0_ps[:], wi_sb[:], u0t_sb[:], start=True, stop=True)
        z_sb = zpool.tile([MLP_W, B], f32)
        nc.vector.tensor_copy(z_sb[:], z0_ps[:])
        readout(z_sb, 0)

        for t in range(NSTEP):
            dx_sb = dxpool.tile([128, B], f32)
            nc.sync.dma_start(dx_sb[:], dxt[t])

            h_ps = psum_h.tile([MLP_W, B], f32)
            nc.tensor.matmul(h_ps[:], w1s_sb[:], z_sb[:], start=True, stop=True)
            h_sb = hpool.tile([MLP_W, B], f32)
            if RELU_ON == "act":
                nc.scalar.activation(h_sb[:], h_ps[:], AF.Relu, bias=b1_sb[:, 0:1])
            else:
                nc.vector.tensor_scalar(
                    h_sb[:], h_ps[:], b1_sb[:, 0:1], 0.0, ALU.add, ALU.max
                )

            f_ps = psum_f.tile([128, NBANK * B], f32)
            for j in range(NBANK):
                nc.tensor.matmul(
                    f_ps[:, j * B : (j + 1) * B], w2_sb[:, j, :], h_sb[:],
                    start=True, stop=True,
                )
            f_sb = fpool.tile([128, NBANK * B], f32)
            for j in range(NBANK):
                nc.scalar.activation(
                    f_sb[:, j * B : (j + 1) * B], f_ps[:, j * B : (j + 1) * B],
                    AF.Tanh, bias=b2_sb[:, j : j + 1],
                )

            g_sb = gpool.tile([128, NBANK * B], f32)
            for j in range(NBANK):
                eng = nc.gpsimd if j < G_ON_GPSIMD else nc.vector
                eng.tensor_mul(
                    g_sb[:, j * B : (j + 1) * B], f_sb[:, j * B : (j + 1) * B],
                    dx_sb[:],
                )
            # einsum reduce: 2 slots of column-tiled matmul pairs
            e_ps = psum_e.tile([MLP_W, B], f32, tag="e_ps")
            for j in range(NBANK):
                half = j % 2  # 0 -> rows 0..63, 1 -> rows 64..127
                nc.tensor.matmul(
                    e_ps[64 * half : 64 * half + 64, :],
                    s_sb[:, j, :],
                    g_sb[:, j * B : (j + 1) * B],
                    start=j < 2, stop=j >= 2,
                    # the sim's zero-region tracker ignores the partition
                    # offset, so the rows-64..127 pair falsely collides
                    # with the rows-0..63 pair
                    skip_group_check=half == 1,
                    tile_position=(0, 64 * half),
                )
            z_prev = z_sb
            z_sb = zpool.tile([MLP_W, B], f32)
            nc.vector.tensor_add(z_sb[:], e_ps[:], z_prev[:])
            readout(z_sb, t + 1)

    nc.compile()
    return nc


def _prep_host(u0, coeffs, W1, b1, W2, b2, Wi, bi, Wr, br):
    f32 = np.float32

    u0t_full = np.empty((IN_CH, B_FULL), f32)
    u0t_full[: IN_CH - 1] = u0.T
    u0t_full[IN_CH - 1] = 1.0

    dX = (coeffs[:, 1:] - coeffs[:, :-1]).astype(f32)  # [B_FULL, NSTEP, IN_CH]
    dxt_small = np.ascontiguousarray(dX.transpose(1, 2, 0))  # [NSTEP, 8, B_FULL]
    dxt_full = np.tile(dxt_small, (1, 128 // IN_CH, 1))

    # stacked mm1 weights: h = W1.T @ z_a + W1.T @ z_b
    w1s = np.zeros((MLP_W, MLP_W), f32)
    w1s[:HID] = W1
    w1s[HID:] = W1

    # init: z_a = z0, z_b = 0
    wi_mat = np.zeros((IN_CH, MLP_W), f32)
    wi_mat[: IN_CH - 1, :HID] = Wi
    wi_mat[IN_CH - 1, :HID] = bi

    w2_banks = np.ascontiguousarray(W2.reshape(MLP_W, NBANK, 128))
    b2_banks = np.ascontiguousarray(b2.reshape(NBANK, 128).T)

    p = np.arange(128)
    s_full = np.zeros((128, NBANK, HID), f32)
    for j in range(NBANK):
        s_full[p, j, 16 * j + p // IN_CH] = 1.0

    wr_stack = np.concatenate([Wr[:, 0], Wr[:, 0]]).reshape(MLP_W, 1).astype(f32)

    return {
        "u0t": u0t_full,
        "dxt": dxt_full,
        "w1s": w1s,
        "b1": np.ascontiguousarray(b1.astype(f32).reshape(MLP_W, 1)),
        "w2": w2_banks.astype(f32),
        "b2": b2_banks.astype(f32),
        "wi": wi_mat,
        "smat": s_full,
        "wrs": wr_stack,
    }


def _make_in_maps(full):
    in_maps = []
    for c in range(N_CORES):
        sl = slice(c * B, (c + 1) * B)
        in_maps.append(
            {
                "u0t": np.ascontiguousarray(full["u0t"][:, sl]),
                "dxt": np.ascontiguousarray(full["dxt"][:, :, sl]),
                "w1s": full["w1s"],
                "b1": full["b1"],
                "w2": full["w2"],
                "b2": full["b2"],
                "wi": full["wi"],
                "smat": full["smat"],
                "wrs": full["wrs"],
            }
        )
    return in_maps


def kernel(u0, coeffs, W1, b1, W2, b2, Wi, bi, Wr, br):
    from concourse.bass_utils import run_bass_kernel_spmd

    br = np.asarray(br, np.float32).reshape(OUT)
    full = _prep_host(
        np.asarray(u0, np.float32), np.asarray(coeffs, np.float32),
        np.asarray(W1, np.float32), np.asarray(b1, np.float32),
        np.asarray(W2, np.float32), np.asarray(b2, np.float32),
        np.asarray(Wi, np.float32), np.asarray(bi, np.float32),
        np.asarray(Wr, np.float32).reshape(HID, OUT), br,
    )
    in_maps = _make_in_maps(full)

    if "nc" not in _CACHE:
        _CACHE["nc"] = _build_bass()
    nc = _CACHE["nc"]

    res = run_bass_kernel_spmd(nc, in_maps, core_ids=list(range(N_CORES)))
    outs = res.results

    out_full = np.empty((B_FULL, T, OUT), np.float32)
    for c in range(N_CORES):
        out_full[c * B : (c + 1) * B, :, 0] = outs[c]["outp"].T
    out_full += br[0]
    return out_full
